# revision 14
# baseline (speedup 1.0000x reference)
"""Expert-parallel MoE SwiGLU kernel for 8 Trainium2 NeuronCores.

Strategy (v8): expert PAIRING with f-split. Experts are sorted by token
load and paired big-with-small; each of the 4 pairs gets 2 cores, each
core computing HALF the f dimension (2048 of 4096) for BOTH experts of
its pair. The host adds the two partial y outputs per expert. Per-core
weight traffic is unchanged (24MB fp16) but the PE streams
384*(Wa+Wb) columns instead of 768*Wmax — routing imbalance only costs
the max-over-pairs instead of max-over-experts.

Per-core schedule: expert A runs the v4 opening (one x DMA per expert,
d-outer first half of group 0 so real matmuls start as weight tiles
land, ~6 warmup dummies to pre-trip the HAM window), then f-major
groups with the previous group's down-projection chains interleaved;
expert B's two groups follow f-major. Down chains lag one group:
A0 downs during A1, A1 downs during B0 (finishing expert A's y, whose
write-out DMAs then overlap B's compute), B0 downs during B1, B1 downs
as the tail. Matmul operands stream fp16 (fp32 PSUM accumulation),
~6e-4 max relative error vs the fp32 reference.
"""

import numpy as np
from contextlib import ExitStack

D_MODEL = 1024
D_FF = 4096
F_HALF = D_FF // 2
N_EXPERTS = 8
N_CORES = 8

_ND = D_MODEL // 128  # 8 contraction chunks over d_model
_FSG = 1024           # f columns per gate/up weight streaming tile
_NFSG = F_HALF // _FSG  # 2 groups per expert per core
_FTG = _FSG // 128    # 8 f-tiles per group

_nc_cache = {}

import os as _os
_CDT = _os.environ.get("MOE_KERNEL_DTYPE", "float16")


def _np_cdt():
    if _CDT == "float16":
        return np.float16
    if _CDT == "bfloat16":
        import ml_dtypes
        return ml_dtypes.bfloat16
    return np.float32


def _build_nc(Wa: int, Wb: int):
    """Per-core Bass program: two experts (cap Wa, Wb) x half-f SwiGLU."""
    import concourse.bacc as bacc
    import concourse.tile as tile
    from concourse import mybir

    f32 = mybir.dt.float32
    f32r = getattr(mybir.dt, _CDT)

    nc = bacc.Bacc("TRN2", target_bir_lowering=False, debug=False,
                   num_devices=N_CORES)
    Wpa = (Wa + 31) // 32 * 32
    Wpb = (Wb + 31) // 32 * 32
    xta = nc.dram_tensor("xta", [128, _ND * Wpa], f32r,
                         kind="ExternalInput").ap()
    xtb = nc.dram_tensor("xtb", [128, _ND * Wpb], f32r,
                         kind="ExternalInput").ap()
    wga = nc.dram_tensor("wga", [_NFSG, _ND, 128, _FSG], f32r,
                         kind="ExternalInput").ap()
    wua = nc.dram_tensor("wua", [_NFSG, _ND, 128, _FSG], f32r,
                         kind="ExternalInput").ap()
    wda = nc.dram_tensor("wda", [F_HALF, D_MODEL], f32r,
                         kind="ExternalInput").ap()
    wgb = nc.dram_tensor("wgb", [_NFSG, _ND, 128, _FSG], f32r,
                         kind="ExternalInput").ap()
    wub = nc.dram_tensor("wub", [_NFSG, _ND, 128, _FSG], f32r,
                         kind="ExternalInput").ap()
    wdb = nc.dram_tensor("wdb", [F_HALF, D_MODEL], f32r,
                         kind="ExternalInput").ap()
    yta = nc.dram_tensor("yta", [D_MODEL, Wa], f32, kind="ExternalOutput").ap()
    ytb = nc.dram_tensor("ytb", [D_MODEL, Wb], f32, kind="ExternalOutput").ap()

    _ctr = [0]

    def _nm():
        _ctr[0] += 1
        return _ctr[0]

    with tile.TileContext(nc) as tc, ExitStack() as ctx:
        xpool = ctx.enter_context(tc.tile_pool(name="x", bufs=1))
        wgp = ctx.enter_context(tc.tile_pool(name="wgp", bufs=4))
        wup = ctx.enter_context(tc.tile_pool(name="wup", bufs=4))
        wdp = ctx.enter_context(tc.tile_pool(name="wdp", bufs=2))
        tp = ctx.enter_context(tc.tile_pool(name="tp", bufs=2))
        gap = ctx.enter_context(tc.tile_pool(name="gap", bufs=3))
        yp = ctx.enter_context(tc.tile_pool(name="yp", bufs=1))
        pp = ctx.enter_context(tc.tile_pool(name="pp", bufs=8, space="PSUM"))

        x_a = xpool.tile([128, _ND * Wpa], f32r, tag="xa", name="x_a")
        x_b = xpool.tile([128, _ND * Wpb], f32r, tag="xb", name="x_b")
        xts_a = [x_a[:, d * Wpa:d * Wpa + Wa] for d in range(_ND)]
        xts_b = [x_b[:, d * Wpb:d * Wpb + Wb] for d in range(_ND)]

        y_acc_a = [yp.tile([128, Wa], f32, tag=f"ya{d}", name=f"y_a{d}")
                   for d in range(_ND)]
        y_acc_b = [yp.tile([128, Wb], f32, tag=f"yb{d}", name=f"y_b{d}")
                   for d in range(_ND)]

        # Warmup dummies: pre-trip the HAM activity window during the
        # DMA lead-in.
        scr_w = xpool.tile([128, 128], f32r, tag="scrw", name="scr_w")
        scr_x = xpool.tile([128, Wa], f32r, tag="scrx", name="scr_x")
        nc.vector.memset(scr_w[:], 0.0)
        nc.vector.memset(scr_x[:], 0.0)
        scr_p = [pp.tile([128, Wa], f32, tag="ps", name=f"scr_p{i}")
                 for i in range(2)]
        for i in range(6):
            nc.tensor.matmul(scr_p[i % 2][:], scr_w[:], scr_x[:],
                             start=True, stop=True)

        # ---- opening DMA order (v4): xa + expert-A group-0 tiles ----
        half = _ND // 2 * Wpa
        nc.sync.dma_start(x_a[:, 0:half], xta[:, 0:half])
        wg0, wu0 = [], []
        for d in range(_ND):
            wg0.append(wgp.tile([128, _FSG], f32r, tag=f"wg{d}",
                                name=f"wgA0_{d}"))
            wu0.append(wup.tile([128, _FSG], f32r, tag=f"wu{d}",
                                name=f"wuA0_{d}"))
        for d in range(3):
            nc.sync.dma_start(wg0[d][:], wga[0, d])
            nc.scalar.dma_start(wu0[d][:], wua[0, d])
        nc.sync.dma_start(x_a[:, half:], xta[:, half:])
        for d in range(3, _ND):
            nc.sync.dma_start(wg0[d][:], wga[0, d])
            nc.scalar.dma_start(wu0[d][:], wua[0, d])
        # expert B's x rides the scalar ring; needed ~half-way in
        nc.scalar.dma_start(x_b[:], xtb[:, :])

        def _swiglu(psg, psu, ft, W):
            g_act = gap.tile([128, W], f32, tag="gact", name=f"ga_{_nm()}")
            nc.scalar.activation(g_act[:], psg[:],
                                 mybir.ActivationFunctionType.Silu)
            t_t = tp.tile([128, W], f32r, tag=f"t{ft}", name=f"t_{_nm()}")
            nc.vector.tensor_mul(t_t[:], g_act[:], psu[:])
            return t_t

        def emit_down(t_tiles, wd_tiles, dts, y_acc, W, first, yt_ap=None):
            # y[dt] (+)= wd[grp rows, dt cols].T @ t ; if yt_ap given this
            # is the expert's last group: write y out, alternating rings
            for dt in dts:
                pdt = pp.tile([128, W], f32, tag="ps", name=f"pd_{_nm()}")
                for ft in range(_FTG):
                    nc.tensor.matmul(
                        pdt[:],
                        wd_tiles[ft][:, dt * 128:(dt + 1) * 128],
                        t_tiles[ft][:],
                        start=(ft == 0), stop=(ft == _FTG - 1))
                if first:
                    nc.vector.tensor_copy(y_acc[dt][:], pdt[:])
                else:
                    nc.vector.tensor_add(y_acc[dt][:], y_acc[dt][:], pdt[:])
                if yt_ap is not None:
                    eng = nc.sync if dt % 2 == 0 else nc.scalar
                    eng.dma_start(yt_ap[dt * 128:(dt + 1) * 128, :],
                                  y_acc[dt][:])

        def emit_group(wg_ap, wu_ap, wd_ap, grp, xts, W, down_cb):
            """f-major group: fetch tiles, 8 gate/up chains, SwiGLU;
            down_cb(ft) interleaves the previous group's down chains."""
            wg_t, wu_t = [], []
            for d in range(_ND):
                g_t = wgp.tile([128, _FSG], f32r, tag=f"wg{d}",
                               name=f"wg_{_nm()}")
                nc.sync.dma_start(g_t[:], wg_ap[grp, d])
                wg_t.append(g_t)
                u_t = wup.tile([128, _FSG], f32r, tag=f"wu{d}",
                               name=f"wu_{_nm()}")
                nc.sync.dma_start(u_t[:], wu_ap[grp, d])
                wu_t.append(u_t)
            t_tiles, wd_tiles = [], []
            for ft in range(_FTG):
                fc = grp * _FTG + ft
                wd_t = wdp.tile([128, D_MODEL], f32r, tag=f"wd{ft}",
                                name=f"wd_{_nm()}")
                nc.sync.dma_start(wd_t[:], wd_ap[fc * 128:(fc + 1) * 128, :])
                wd_tiles.append(wd_t)
                psg = pp.tile([128, W], f32, tag="ps", name=f"pg_{_nm()}")
                for d in range(_ND):
                    nc.tensor.matmul(
                        psg[:], wg_t[d][:, ft * 128:(ft + 1) * 128],
                        xts[d], start=(d == 0), stop=(d == _ND - 1))
                psu = pp.tile([128, W], f32, tag="ps", name=f"pu_{_nm()}")
                for d in range(_ND):
                    nc.tensor.matmul(
                        psu[:], wu_t[d][:, ft * 128:(ft + 1) * 128],
                        xts[d], start=(d == 0), stop=(d == _ND - 1))
                t_tiles.append(_swiglu(psg, psu, ft, W))
                down_cb(ft)
            return t_tiles, wd_tiles

        # ---------------- expert A, group 0 (v4 opening) --------------
        psg0 = [pp.tile([128, Wa], f32, tag="ps", name=f"pg0_{i}")
                for i in range(4)]
        psu0 = [pp.tile([128, Wa], f32, tag="ps", name=f"pu0_{i}")
                for i in range(4)]
        for d in range(_ND):
            for ft in range(4):
                nc.tensor.matmul(
                    psg0[ft][:], wg0[d][:, ft * 128:(ft + 1) * 128],
                    xts_a[d], start=(d == 0), stop=(d == _ND - 1))
                nc.tensor.matmul(
                    psu0[ft][:], wu0[d][:, ft * 128:(ft + 1) * 128],
                    xts_a[d], start=(d == 0), stop=(d == _ND - 1))
        t_a0 = [_swiglu(psg0[ft], psu0[ft], ft, Wa) for ft in range(4)]

        wd_a0 = []
        for ft in range(_FTG):
            wd_t = wdp.tile([128, D_MODEL], f32r, tag=f"wd{ft}",
                            name=f"wdA0_{ft}")
            nc.sync.dma_start(wd_t[:], wda[ft * 128:(ft + 1) * 128, :])
            wd_a0.append(wd_t)

        for ft in range(4, _FTG):
            psg = pp.tile([128, Wa], f32, tag="ps", name=f"pg_{_nm()}")
            for d in range(_ND):
                nc.tensor.matmul(
                    psg[:], wg0[d][:, ft * 128:(ft + 1) * 128], xts_a[d],
                    start=(d == 0), stop=(d == _ND - 1))
            psu = pp.tile([128, Wa], f32, tag="ps", name=f"pu_{_nm()}")
            for d in range(_ND):
                nc.tensor.matmul(
                    psu[:], wu0[d][:, ft * 128:(ft + 1) * 128], xts_a[d],
                    start=(d == 0), stop=(d == _ND - 1))
            t_a0.append(_swiglu(psg, psu, ft, Wa))

        # ---------------- expert A, group 1 (downs of A0) --------------
        t_a1, wd_a1 = emit_group(
            wga, wua, wda, 1, xts_a, Wa,
            lambda ft: emit_down(t_a0, wd_a0, (ft,), y_acc_a, Wa,
                                 first=True))

        # -------- expert B, group 0 (downs of A1 finish expert A) ------
        t_b0, wd_b0 = emit_group(
            wgb, wub, wdb, 0, xts_b, Wb,
            lambda ft: emit_down(t_a1, wd_a1, (ft,), y_acc_a, Wa,
                                 first=False, yt_ap=yta))

        # ---------------- expert B, group 1 (downs of B0) --------------
        t_b1, wd_b1 = emit_group(
            wgb, wub, wdb, 1, xts_b, Wb,
            lambda ft: emit_down(t_b0, wd_b0, (ft,), y_acc_b, Wb,
                                 first=True))

        # ---------------- tail: downs of B1 ----------------------------
        emit_down(t_b1, wd_b1, range(_ND), y_acc_b, Wb, first=False,
                  yt_ap=ytb)

    nc.compile()
    return nc


def _pack_gu_half(w, h):
    # [D, F] half h -> [NFSG, ND, 128, FSG], each streamed tile contiguous
    w = np.asarray(w)[:, h * F_HALF:(h + 1) * F_HALF].astype(_np_cdt())
    return np.ascontiguousarray(
        w.reshape(_ND, 128, _NFSG, _FSG).transpose(2, 0, 1, 3))


def _pack_x(x_flat, toks, W):
    Wp = (W + 31) // 32 * 32
    xt_e = np.zeros((128, _ND * Wp), dtype=_np_cdt())
    xe = x_flat[toks].T.astype(_np_cdt())
    n = len(toks)
    for d in range(_ND):
        xt_e[:, d * Wp:d * Wp + n] = xe[d * 128:(d + 1) * 128, :]
    return xt_e


def _prepare(x, expert_idx, w_gate, w_up, w_down):
    """Pair experts, build/fetch the program, build per-core in_maps.

    Returns (nc, in_maps, meta); meta carries what's needed to unshard.
    """
    x = np.asarray(x, dtype=np.float32)
    idx = np.asarray(expert_idx).astype(np.int64)
    B, S, D = x.shape
    T = B * S
    x_flat = np.ascontiguousarray(x.reshape(T, D))
    idx_flat = idx.reshape(T)

    tok_lists = [np.nonzero(idx_flat == e)[0] for e in range(N_EXPERTS)]
    loads = np.array([len(t) for t in tok_lists])
    order = np.argsort(-loads, kind="stable")
    Wa = max(128, int(loads[order[0]]))
    Wb = max(128, int(loads[order[4]]))

    key = (Wa, Wb)
    if key not in _nc_cache:
        _nc_cache[key] = _build_nc(Wa, Wb)
    nc = _nc_cache[key]

    wdt = [np.asarray(w_down[e]).astype(_np_cdt()) for e in range(N_EXPERTS)]
    in_maps = []
    pairs = []
    for p in range(4):
        eA, eB = int(order[p]), int(order[7 - p])
        pairs.append((eA, eB))
        xa = _pack_x(x_flat, tok_lists[eA], Wa)
        xb = _pack_x(x_flat, tok_lists[eB], Wb)
        for h in range(2):
            in_maps.append({
                "xta": xa,
                "xtb": xb,
                "wga": _pack_gu_half(w_gate[eA], h),
                "wua": _pack_gu_half(w_up[eA], h),
                "wda": np.ascontiguousarray(
                    wdt[eA][h * F_HALF:(h + 1) * F_HALF, :]),
                "wgb": _pack_gu_half(w_gate[eB], h),
                "wub": _pack_gu_half(w_up[eB], h),
                "wdb": np.ascontiguousarray(
                    wdt[eB][h * F_HALF:(h + 1) * F_HALF, :]),
            })
    meta = dict(tok_lists=tok_lists, pairs=pairs, shape=(B, S, D), T=T)
    return nc, in_maps, meta


def _combine(res, meta):
    tok_lists, pairs = meta["tok_lists"], meta["pairs"]
    B, S, D = meta["shape"]
    out_flat = np.zeros((meta["T"], D), dtype=np.float32)
    for p, (eA, eB) in enumerate(pairs):
        ra, rb = res.results[2 * p], res.results[2 * p + 1]
        for e, kk in ((eA, "yta"), (eB, "ytb")):
            toks = tok_lists[e]
            y = ra[kk] + rb[kk]
            out_flat[toks] = y[:, :len(toks)].T
    return out_flat.reshape(B, S, D)


def kernel(x, expert_idx, w_gate, w_up, w_down):
    from concourse.bass_utils import run_bass_kernel_spmd

    idx = np.asarray(expert_idx).astype(np.int64)
    T = idx.size
    idx_flat = idx.reshape(T)
    cap = max(1, int(np.bincount(idx_flat, minlength=N_EXPERTS).max()))
    if cap > 512:
        # extreme routing imbalance: fall back to the single-expert-
        # per-core kernel processing rounds of <=512 tokens per expert
        import kernel_v4_fallback as _fb  # pragma: no cover
        return _fb.kernel(x, expert_idx, w_gate, w_up, w_down)

    nc, in_maps, meta = _prepare(x, expert_idx, w_gate, w_up, w_down)
    res = None
    for attempt in range(3):
        try:
            res = run_bass_kernel_spmd(nc, in_maps,
                                       core_ids=list(range(N_CORES)))
            break
        except Exception:
            if attempt == 2:
                raise
            import time
            time.sleep(3.0)
            try:
                import jax
                jax.clear_caches()
                jax.clear_backends()
            except Exception:
                pass
    return _combine(res, meta)


# revision 15
# speedup vs baseline: 1.0296x; 1.0296x over previous
"""Expert-parallel MoE SwiGLU kernel for 8 Trainium2 NeuronCores.

Strategy (v8): expert PAIRING with f-split. Experts are sorted by token
load and paired big-with-small; each of the 4 pairs gets 2 cores, each
core computing HALF the f dimension (2048 of 4096) for BOTH experts of
its pair. The host adds the two partial y outputs per expert. Per-core
weight traffic is unchanged (24MB fp16) but the PE streams
384*(Wa+Wb) columns instead of 768*Wmax — routing imbalance only costs
the max-over-pairs instead of max-over-experts.

Per-core schedule: expert A runs the v4 opening (one x DMA per expert,
d-outer first half of group 0 so real matmuls start as weight tiles
land, ~6 warmup dummies to pre-trip the HAM window), then f-major
groups with the previous group's down-projection chains interleaved;
expert B's two groups follow f-major. Down chains lag one group:
A0 downs during A1, A1 downs during B0 (finishing expert A's y, whose
write-out DMAs then overlap B's compute), B0 downs during B1, B1 downs
as the tail. Matmul operands stream fp16 (fp32 PSUM accumulation),
~6e-4 max relative error vs the fp32 reference.
"""

import numpy as np
from contextlib import ExitStack

D_MODEL = 1024
D_FF = 4096
F_HALF = D_FF // 2
N_EXPERTS = 8
N_CORES = 8

_ND = D_MODEL // 128  # 8 contraction chunks over d_model
_FSG = 1024           # f columns per gate/up weight streaming tile
_NFSG = F_HALF // _FSG  # 2 groups per expert per core
_FTG = _FSG // 128    # 8 f-tiles per group

_nc_cache = {}

import os as _os
_CDT = _os.environ.get("MOE_KERNEL_DTYPE", "float16")


def _np_cdt():
    if _CDT == "float16":
        return np.float16
    if _CDT == "bfloat16":
        import ml_dtypes
        return ml_dtypes.bfloat16
    return np.float32


def _build_nc(Wa: int, Wb: int):
    """Per-core Bass program: two experts (cap Wa, Wb) x half-f SwiGLU."""
    import concourse.bacc as bacc
    import concourse.tile as tile
    from concourse import mybir

    f32 = mybir.dt.float32
    f32r = getattr(mybir.dt, _CDT)

    nc = bacc.Bacc("TRN2", target_bir_lowering=False, debug=False,
                   num_devices=N_CORES)
    Wpa = (Wa + 31) // 32 * 32
    Wpb = (Wb + 31) // 32 * 32
    xta = nc.dram_tensor("xta", [128, _ND * Wpa], f32r,
                         kind="ExternalInput").ap()
    xtb = nc.dram_tensor("xtb", [128, _ND * Wpb], f32r,
                         kind="ExternalInput").ap()
    wga = nc.dram_tensor("wga", [_NFSG, _ND, 128, _FSG], f32r,
                         kind="ExternalInput").ap()
    wua = nc.dram_tensor("wua", [_NFSG, _ND, 128, _FSG], f32r,
                         kind="ExternalInput").ap()
    wda = nc.dram_tensor("wda", [F_HALF, D_MODEL], f32r,
                         kind="ExternalInput").ap()
    wgb = nc.dram_tensor("wgb", [_NFSG, _ND, 128, _FSG], f32r,
                         kind="ExternalInput").ap()
    wub = nc.dram_tensor("wub", [_NFSG, _ND, 128, _FSG], f32r,
                         kind="ExternalInput").ap()
    wdb = nc.dram_tensor("wdb", [F_HALF, D_MODEL], f32r,
                         kind="ExternalInput").ap()
    yta = nc.dram_tensor("yta", [D_MODEL, Wa], f32r,
                         kind="ExternalOutput").ap()
    ytb = nc.dram_tensor("ytb", [D_MODEL, Wb], f32r,
                         kind="ExternalOutput").ap()

    _ctr = [0]

    def _nm():
        _ctr[0] += 1
        return _ctr[0]

    with tile.TileContext(nc) as tc, ExitStack() as ctx:
        xpool = ctx.enter_context(tc.tile_pool(name="x", bufs=1))
        wgp = ctx.enter_context(tc.tile_pool(name="wgp", bufs=4))
        wup = ctx.enter_context(tc.tile_pool(name="wup", bufs=4))
        wdp = ctx.enter_context(tc.tile_pool(name="wdp", bufs=3))
        tp = ctx.enter_context(tc.tile_pool(name="tp", bufs=2))
        gap = ctx.enter_context(tc.tile_pool(name="gap", bufs=3))
        yp = ctx.enter_context(tc.tile_pool(name="yp", bufs=1))
        pp = ctx.enter_context(tc.tile_pool(name="pp", bufs=8, space="PSUM"))

        x_a = xpool.tile([128, _ND * Wpa], f32r, tag="xa", name="x_a")
        x_b = xpool.tile([128, _ND * Wpb], f32r, tag="xb", name="x_b")
        xts_a = [x_a[:, d * Wpa:d * Wpa + Wa] for d in range(_ND)]
        xts_b = [x_b[:, d * Wpb:d * Wpb + Wb] for d in range(_ND)]

        y_acc_a = [yp.tile([128, Wa], f32r, tag=f"ya{d}", name=f"y_a{d}")
                   for d in range(_ND)]
        y_acc_b = [yp.tile([128, Wb], f32r, tag=f"yb{d}", name=f"y_b{d}")
                   for d in range(_ND)]

        # Warmup dummies: pre-trip the HAM activity window during the
        # DMA lead-in.
        scr_w = xpool.tile([128, 128], f32r, tag="scrw", name="scr_w")
        scr_x = xpool.tile([128, Wa], f32r, tag="scrx", name="scr_x")
        nc.vector.memset(scr_w[:], 0.0)
        nc.vector.memset(scr_x[:], 0.0)
        scr_p = [pp.tile([128, Wa], f32, tag="ps", name=f"scr_p{i}")
                 for i in range(2)]
        for i in range(6):
            nc.tensor.matmul(scr_p[i % 2][:], scr_w[:], scr_x[:],
                             start=True, stop=True)

        # ---- opening DMA order (v4): xa + expert-A group-0 tiles ----
        half = _ND // 2 * Wpa
        nc.sync.dma_start(x_a[:, 0:half], xta[:, 0:half])
        wg0, wu0 = [], []
        for d in range(_ND):
            wg0.append(wgp.tile([128, _FSG], f32r, tag=f"wg{d}",
                                name=f"wgA0_{d}"))
            wu0.append(wup.tile([128, _FSG], f32r, tag=f"wu{d}",
                                name=f"wuA0_{d}"))
        for d in range(3):
            nc.sync.dma_start(wg0[d][:], wga[0, d])
            nc.scalar.dma_start(wu0[d][:], wua[0, d])
        nc.sync.dma_start(x_a[:, half:], xta[:, half:])
        for d in range(3, _ND):
            nc.sync.dma_start(wg0[d][:], wga[0, d])
            nc.scalar.dma_start(wu0[d][:], wua[0, d])

        def _swiglu(psg, psu, ft, W):
            g_act = gap.tile([128, W], f32, tag="gact", name=f"ga_{_nm()}")
            nc.scalar.activation(g_act[:], psg[:],
                                 mybir.ActivationFunctionType.Silu)
            t_t = tp.tile([128, W], f32r, tag=f"t{ft}", name=f"t_{_nm()}")
            nc.vector.tensor_mul(t_t[:], g_act[:], psu[:])
            return t_t

        def emit_down(t_tiles, wd_tiles, dts, y_acc, W, first, yt_ap=None):
            # y[dt] (+)= wd[grp rows, dt cols].T @ t ; if yt_ap given this
            # is the expert's last group: write y out, alternating rings
            for dt in dts:
                pdt = pp.tile([128, W], f32, tag="ps", name=f"pd_{_nm()}")
                for ft in range(_FTG):
                    nc.tensor.matmul(
                        pdt[:],
                        wd_tiles[ft][:, dt * 128:(dt + 1) * 128],
                        t_tiles[ft][:],
                        start=(ft == 0), stop=(ft == _FTG - 1))
                if first:
                    nc.vector.tensor_copy(y_acc[dt][:], pdt[:])
                else:
                    nc.vector.tensor_add(y_acc[dt][:], y_acc[dt][:], pdt[:])
                if yt_ap is not None:
                    eng = nc.sync if dt % 2 == 0 else nc.scalar
                    eng.dma_start(yt_ap[dt * 128:(dt + 1) * 128, :],
                                  y_acc[dt][:])

        def emit_group(wg_ap, wu_ap, wd_ap, grp, xts, W, down_cb):
            """f-major group: fetch tiles, 8 gate/up chains, SwiGLU;
            down_cb(ft) interleaves the previous group's down chains."""
            wg_t, wu_t = [], []
            for d in range(_ND):
                g_t = wgp.tile([128, _FSG], f32r, tag=f"wg{d}",
                               name=f"wg_{_nm()}")
                nc.sync.dma_start(g_t[:], wg_ap[grp, d])
                wg_t.append(g_t)
                u_t = wup.tile([128, _FSG], f32r, tag=f"wu{d}",
                               name=f"wu_{_nm()}")
                nc.sync.dma_start(u_t[:], wu_ap[grp, d])
                wu_t.append(u_t)
            t_tiles, wd_tiles = [], []
            for ft in range(_FTG):
                fc = grp * _FTG + ft
                wd_t = wdp.tile([128, D_MODEL], f32r, tag=f"wd{ft}",
                                name=f"wd_{_nm()}")
                nc.sync.dma_start(wd_t[:], wd_ap[fc * 128:(fc + 1) * 128, :])
                wd_tiles.append(wd_t)
                psg = pp.tile([128, W], f32, tag="ps", name=f"pg_{_nm()}")
                for d in range(_ND):
                    nc.tensor.matmul(
                        psg[:], wg_t[d][:, ft * 128:(ft + 1) * 128],
                        xts[d], start=(d == 0), stop=(d == _ND - 1))
                psu = pp.tile([128, W], f32, tag="ps", name=f"pu_{_nm()}")
                for d in range(_ND):
                    nc.tensor.matmul(
                        psu[:], wu_t[d][:, ft * 128:(ft + 1) * 128],
                        xts[d], start=(d == 0), stop=(d == _ND - 1))
                t_tiles.append(_swiglu(psg, psu, ft, W))
                down_cb(ft)
            return t_tiles, wd_tiles

        # ---------------- expert A, group 0 (v4 opening) --------------
        psg0 = [pp.tile([128, Wa], f32, tag="ps", name=f"pg0_{i}")
                for i in range(4)]
        psu0 = [pp.tile([128, Wa], f32, tag="ps", name=f"pu0_{i}")
                for i in range(4)]
        for d in range(_ND):
            for ft in range(4):
                nc.tensor.matmul(
                    psg0[ft][:], wg0[d][:, ft * 128:(ft + 1) * 128],
                    xts_a[d], start=(d == 0), stop=(d == _ND - 1))
                nc.tensor.matmul(
                    psu0[ft][:], wu0[d][:, ft * 128:(ft + 1) * 128],
                    xts_a[d], start=(d == 0), stop=(d == _ND - 1))
        t_a0 = [_swiglu(psg0[ft], psu0[ft], ft, Wa) for ft in range(4)]

        wd_a0 = []
        for ft in range(_FTG):
            wd_t = wdp.tile([128, D_MODEL], f32r, tag=f"wd{ft}",
                            name=f"wdA0_{ft}")
            nc.sync.dma_start(wd_t[:], wda[ft * 128:(ft + 1) * 128, :])
            wd_a0.append(wd_t)
        # expert B's x rides the scalar ring; needed ~half-way in
        nc.scalar.dma_start(x_b[:], xtb[:, :])

        for ft in range(4, _FTG):
            psg = pp.tile([128, Wa], f32, tag="ps", name=f"pg_{_nm()}")
            for d in range(_ND):
                nc.tensor.matmul(
                    psg[:], wg0[d][:, ft * 128:(ft + 1) * 128], xts_a[d],
                    start=(d == 0), stop=(d == _ND - 1))
            psu = pp.tile([128, Wa], f32, tag="ps", name=f"pu_{_nm()}")
            for d in range(_ND):
                nc.tensor.matmul(
                    psu[:], wu0[d][:, ft * 128:(ft + 1) * 128], xts_a[d],
                    start=(d == 0), stop=(d == _ND - 1))
            t_a0.append(_swiglu(psg, psu, ft, Wa))

        # ---------------- expert A, group 1 (downs of A0) --------------
        t_a1, wd_a1 = emit_group(
            wga, wua, wda, 1, xts_a, Wa,
            lambda ft: emit_down(t_a0, wd_a0, (ft,), y_acc_a, Wa,
                                 first=True))

        # -------- expert B, group 0 (downs of A1 finish expert A) ------
        t_b0, wd_b0 = emit_group(
            wgb, wub, wdb, 0, xts_b, Wb,
            lambda ft: emit_down(t_a1, wd_a1, (ft,), y_acc_a, Wa,
                                 first=False, yt_ap=yta))

        # ---------------- expert B, group 1 (downs of B0) --------------
        t_b1, wd_b1 = emit_group(
            wgb, wub, wdb, 1, xts_b, Wb,
            lambda ft: emit_down(t_b0, wd_b0, (ft,), y_acc_b, Wb,
                                 first=True))

        # ---------------- tail: downs of B1 ----------------------------
        emit_down(t_b1, wd_b1, range(_ND), y_acc_b, Wb, first=False,
                  yt_ap=ytb)

    nc.compile()
    return nc


def _pack_gu_half(w, h):
    # [D, F] half h -> [NFSG, ND, 128, FSG], each streamed tile contiguous
    w = np.asarray(w)[:, h * F_HALF:(h + 1) * F_HALF].astype(_np_cdt())
    return np.ascontiguousarray(
        w.reshape(_ND, 128, _NFSG, _FSG).transpose(2, 0, 1, 3))


def _pack_x(x_flat, toks, W):
    Wp = (W + 31) // 32 * 32
    xt_e = np.zeros((128, _ND * Wp), dtype=_np_cdt())
    xe = x_flat[toks].T.astype(_np_cdt())
    n = len(toks)
    for d in range(_ND):
        xt_e[:, d * Wp:d * Wp + n] = xe[d * 128:(d + 1) * 128, :]
    return xt_e


def _prepare(x, expert_idx, w_gate, w_up, w_down):
    """Pair experts, build/fetch the program, build per-core in_maps.

    Returns (nc, in_maps, meta); meta carries what's needed to unshard.
    """
    x = np.asarray(x, dtype=np.float32)
    idx = np.asarray(expert_idx).astype(np.int64)
    B, S, D = x.shape
    T = B * S
    x_flat = np.ascontiguousarray(x.reshape(T, D))
    idx_flat = idx.reshape(T)

    tok_lists = [np.nonzero(idx_flat == e)[0] for e in range(N_EXPERTS)]
    loads = np.array([len(t) for t in tok_lists])
    order = np.argsort(-loads, kind="stable")
    Wa = max(128, int(loads[order[0]]))
    Wb = max(128, int(loads[order[4]]))

    key = (Wa, Wb)
    if key not in _nc_cache:
        _nc_cache[key] = _build_nc(Wa, Wb)
    nc = _nc_cache[key]

    wdt = [np.asarray(w_down[e]).astype(_np_cdt()) for e in range(N_EXPERTS)]
    in_maps = []
    pairs = []
    for p in range(4):
        eA, eB = int(order[p]), int(order[7 - p])
        pairs.append((eA, eB))
        xa = _pack_x(x_flat, tok_lists[eA], Wa)
        xb = _pack_x(x_flat, tok_lists[eB], Wb)
        for h in range(2):
            in_maps.append({
                "xta": xa,
                "xtb": xb,
                "wga": _pack_gu_half(w_gate[eA], h),
                "wua": _pack_gu_half(w_up[eA], h),
                "wda": np.ascontiguousarray(
                    wdt[eA][h * F_HALF:(h + 1) * F_HALF, :]),
                "wgb": _pack_gu_half(w_gate[eB], h),
                "wub": _pack_gu_half(w_up[eB], h),
                "wdb": np.ascontiguousarray(
                    wdt[eB][h * F_HALF:(h + 1) * F_HALF, :]),
            })
    meta = dict(tok_lists=tok_lists, pairs=pairs, shape=(B, S, D), T=T)
    return nc, in_maps, meta


def _combine(res, meta):
    tok_lists, pairs = meta["tok_lists"], meta["pairs"]
    B, S, D = meta["shape"]
    out_flat = np.zeros((meta["T"], D), dtype=np.float32)
    for p, (eA, eB) in enumerate(pairs):
        ra, rb = res.results[2 * p], res.results[2 * p + 1]
        for e, kk in ((eA, "yta"), (eB, "ytb")):
            toks = tok_lists[e]
            y = ra[kk].astype(np.float32) + rb[kk].astype(np.float32)
            out_flat[toks] = y[:, :len(toks)].T
    return out_flat.reshape(B, S, D)


def kernel(x, expert_idx, w_gate, w_up, w_down):
    from concourse.bass_utils import run_bass_kernel_spmd

    idx = np.asarray(expert_idx).astype(np.int64)
    T = idx.size
    idx_flat = idx.reshape(T)
    cap = max(1, int(np.bincount(idx_flat, minlength=N_EXPERTS).max()))
    if cap > 512:
        # extreme routing imbalance: fall back to the single-expert-
        # per-core kernel processing rounds of <=512 tokens per expert
        import kernel_v4_fallback as _fb  # pragma: no cover
        return _fb.kernel(x, expert_idx, w_gate, w_up, w_down)

    nc, in_maps, meta = _prepare(x, expert_idx, w_gate, w_up, w_down)
    res = None
    for attempt in range(3):
        try:
            res = run_bass_kernel_spmd(nc, in_maps,
                                       core_ids=list(range(N_CORES)))
            break
        except Exception:
            if attempt == 2:
                raise
            import time
            time.sleep(3.0)
            try:
                import jax
                jax.clear_caches()
                jax.clear_backends()
            except Exception:
                pass
    return _combine(res, meta)
